# revision 20
# baseline (speedup 1.0000x reference)
"""Trainium2 Bass kernel for nn_CrossAttention (linear cross-attention block).

Computation (per batch b):
  xn  = LN(x[b]; norm_g, norm_b)                 [T, D]
  xfn = LN(xf[b]; tnorm_g, tnorm_b)              [N, TD]
  q   = softmax_c((xn @ Wq + bq).reshape(T,H,C))
  k   = softmax_n((xfn @ Wk + bk).reshape(N,H,C))
  v   = (xfn @ Wv + bv).reshape(N,H,C)
  attn= einsum('nhc,nhd->hcd', k, v); y = einsum('thc,hcd->thd', q, attn)
  e   = silu(emb) @ emb_W + emb_b; scale, shift = split(e)
  h   = LN(y; fnorm_g, fnorm_b) * (1+scale) + shift
  out = x + silu(h) @ out_W + out_b

Sharding: pure data-parallel over batch B=32 across 8 NeuronCores (4 each).

Device strategy: the whole middle section (Q-softmax -> y -> LN -> FiLM ->
silu) runs in TRANSPOSED layout [d-part, t-free], so no per-tile SBUF
transposes are needed:
  - qT comes straight out of the Q projection (lhsT=Wq, rhs=xnT);
  - the q-softmax denominator is broadcast with a block-diagonal ones
    matmul (one [128,128] matmul per tile) + DVE reciprocal;
  - y is produced transposed by using the block-diagonal attn tile as lhsT;
  - LN stats are partition-sums via ones-column matmuls; rstd via the
    scalar engine's Abs_reciprocal_sqrt; mean/rstd rows broadcast across
    partitions by gpsimd;
  - FiLM scale/bias are per-partition columns in this layout, fused into a
    single Silu activation (scale=A, bias=B);
  - silu(h) lands transposed = exactly the lhsT the out-projection needs.
x enters only via 8 big DMA transposes per batch (raw bf16); LN of x/xf is
applied in transposed space (gain/bias pre-folded into W/proj biases on the
host).
"""

from contextlib import ExitStack

import numpy as np
import ml_dtypes

import concourse.bass as bass
import concourse.mybir as mybir
import concourse.tile as tile
from concourse import bacc
from concourse.bass_utils import run_bass_kernel_spmd
from concourse.masks import make_identity

# problem shapes (hardcoded per contract)
B, T, N, D, TD, H, C, TE = 32, 1024, 256, 1024, 768, 16, 64, 2048
D2 = 2 * D
EPS = 1e-5
NCORES = 8
BPC = B // NCORES           # batches per core
TI = T // 128               # 8 t-tiles
KD = D // 128               # 8 k-tiles over D
KTD = TD // 128             # 6 k-tiles over TD
KTE = TE // 128             # 16 k-tiles over TE
NT = N // 128               # 2 n-tiles
NCH = D // 512              # 2 free 512-chunks over D

F32 = mybir.dt.float32
BF16 = mybir.dt.bfloat16
AF = mybir.ActivationFunctionType
ALU = mybir.AluOpType
NBF = ml_dtypes.bfloat16

_PROGRAM = None  # cached (nc) build


def _build_program():
    nc = bacc.Bacc("TRN2", target_bir_lowering=False, debug=False,
                   num_devices=NCORES)

    # ---- DRAM I/O ----
    d_xbf = nc.dram_tensor("xbf", [BPC, T, D], BF16, kind="ExternalInput")
    d_xf32 = nc.dram_tensor("xf32", [BPC, T, D], F32, kind="ExternalInput")
    d_xfbf = nc.dram_tensor("xfbf", [BPC, N, TD], BF16, kind="ExternalInput")
    d_emb = nc.dram_tensor("emb", [BPC, TE], F32, kind="ExternalInput")
    d_wq = nc.dram_tensor("wq", [D, D], BF16, kind="ExternalInput")
    d_wk = nc.dram_tensor("wk", [TD, D], BF16, kind="ExternalInput")
    d_wv = nc.dram_tensor("wv", [TD, D], BF16, kind="ExternalInput")
    d_wo = nc.dram_tensor("wo", [D, D], BF16, kind="ExternalInput")
    d_wemb = nc.dram_tensor("wemb", [TE, D2], BF16, kind="ExternalInput")
    d_bqc = nc.dram_tensor("bqc", [128, KD], F32, kind="ExternalInput")
    d_bke = nc.dram_tensor("bke", [D], BF16, kind="ExternalInput")
    d_bve = nc.dram_tensor("bve", [D], BF16, kind="ExternalInput")
    d_outb = nc.dram_tensor("outb", [D], BF16, kind="ExternalInput")
    d_embb = nc.dram_tensor("embb", [D2], BF16, kind="ExternalInput")
    d_fgc = nc.dram_tensor("fgc", [128, KD], F32, kind="ExternalInput")
    d_fbc = nc.dram_tensor("fbc", [128, KD], F32, kind="ExternalInput")
    d_sel16 = nc.dram_tensor("sel16", [128, KD, H], BF16, kind="ExternalInput")
    d_pick = nc.dram_tensor("pick", [H, KD, 128], BF16, kind="ExternalInput")
    d_out = nc.dram_tensor("out", [BPC, T, D], F32, kind="ExternalOutput")

    with tile.TileContext(nc) as tc, ExitStack() as ctx:
        wpool = ctx.enter_context(tc.tile_pool(name="weights", bufs=1))
        cpool = ctx.enter_context(tc.tile_pool(name="consts", bufs=1))

        # ---- persistent weights ----
        wq_sb = wpool.tile([128, KD, D], BF16)
        nc.scalar.dma_start(wq_sb[:], d_wq[:].rearrange("(i p) m -> p i m", p=128))
        wk_sb = wpool.tile([128, KTD, D], BF16)
        nc.scalar.dma_start(wk_sb[:], d_wk[:].rearrange("(i p) m -> p i m", p=128))
        wv_sb = wpool.tile([128, KTD, D], BF16)
        nc.scalar.dma_start(wv_sb[:], d_wv[:].rearrange("(i p) m -> p i m", p=128))
        wo_sb = wpool.tile([128, KD, D], BF16)
        nc.scalar.dma_start(wo_sb[:], d_wo[:].rearrange("(i p) m -> p i m", p=128))

        # ---- constants ----
        bqc = cpool.tile([128, KD], F32)
        nc.scalar.dma_start(bqc[:], d_bqc[:])
        sel16 = cpool.tile([128, KD, H], BF16)
        nc.scalar.dma_start(sel16[:], d_sel16[:])
        pick = cpool.tile([H, KD, 128], BF16)
        nc.scalar.dma_start(pick[:], d_pick[:])
        bke_r = cpool.tile([1, D], BF16)
        nc.scalar.dma_start(bke_r[:], d_bke[None, :])
        bve_r = cpool.tile([1, D], BF16)
        nc.scalar.dma_start(bve_r[:], d_bve[None, :])
        outb_r = cpool.tile([1, D], BF16)
        nc.scalar.dma_start(outb_r[:], d_outb[None, :])
        ones_r = cpool.tile([1, 128], BF16)
        nc.vector.memset(ones_r[:], 1.0)
        ones_c = cpool.tile([128, 1], BF16)
        nc.vector.memset(ones_c[:], 1.0)
        eps_c = cpool.tile([128, 1], F32)
        nc.vector.memset(eps_c[:], EPS)
        ident = cpool.tile([128, 128], BF16)
        make_identity(nc, ident[:])
        identf = cpool.tile([4, 4], F32)
        make_identity(nc, identf[:])

        a_col = cpool.tile([128, KD, BPC], F32)   # FiLM A columns
        b_col = cpool.tile([128, KD, BPC], F32)   # FiLM B columns

        # ---- emb / FiLM phase (all BPC batches at once) ----
        with tc.tile_pool(name="wemb", bufs=1) as wep, \
             tc.tile_pool(name="etmp", bufs=1) as ep, \
             tc.tile_pool(name="pse", bufs=2, space=bass.MemorySpace.PSUM) as pse:
            wemb_sb = wep.tile([128, KTE, D2], BF16)
            embb_r = ep.tile([1, D2], BF16)
            nc.scalar.dma_start(embb_r[:], d_embb[None, :])
            fg_c = ep.tile([128, KD], F32)
            nc.scalar.dma_start(fg_c[:], d_fgc[:])
            fb_c = ep.tile([128, KD], F32)
            nc.scalar.dma_start(fb_c[:], d_fbc[:])
            nc.scalar.dma_start(wemb_sb[:],
                              d_wemb[:].rearrange("(i p) m -> p i m", p=128))
            emb_sb = ep.tile([BPC, TE], F32)
            nc.scalar.dma_start(emb_sb[:], d_emb[:])
            semb = ep.tile([BPC, TE], BF16)
            nc.scalar.activation(semb[:], emb_sb[:], AF.Silu)
            embT = ep.tile([128, KTE, BPC], BF16)
            for c in range(KTE):
                pst = pse.tile([128, BPC], BF16, tag="pst")
                nc.tensor.transpose(pst[:], semb[:, c * 128:(c + 1) * 128],
                                    ident[0:BPC, 0:BPC])
                nc.vector.tensor_copy(embT[:, c, :], pst[:])
            e_sb = ep.tile([BPC, D2], F32)
            for chn in range(D2 // 512):
                pe = pse.tile([BPC, 512], F32, tag="pe")
                for kt in range(KTE):
                    nc.tensor.matmul(pe[:], embT[:, kt, :],
                                     wemb_sb[:, kt, chn * 512:(chn + 1) * 512],
                                     start=(kt == 0), stop=False)
                nc.tensor.matmul(pe[:], ones_r[0:1, 0:BPC],
                                 embb_r[0:1, chn * 512:(chn + 1) * 512],
                                 start=False, stop=True)
                nc.vector.tensor_copy(e_sb[:, chn * 512:(chn + 1) * 512], pe[:])
            # transpose scale/shift to columns, build A/B FiLM columns
            for j in range(KD):
                js = slice(j * 128, (j + 1) * 128)
                pts = pse.tile([128, BPC], F32, tag="pts")
                nc.tensor.transpose(pts[:], e_sb[0:BPC, js], identf[:])
                sT = ep.tile([128, BPC], F32, tag="sT")
                nc.vector.tensor_copy(sT[:], pts[:])
                pth = pse.tile([128, BPC], F32, tag="pts")
                nc.tensor.transpose(pth[:], e_sb[0:BPC, D + j * 128:D + (j + 1) * 128],
                                    identf[:])
                hT = ep.tile([128, BPC], F32, tag="hT")
                nc.vector.tensor_copy(hT[:], pth[:])
                nc.vector.tensor_scalar(a_col[:, j, :], sT[:], 1.0,
                                        fg_c[:, j:j + 1], ALU.add, ALU.mult)
                tmb = ep.tile([128, BPC], F32, tag="tmb")
                nc.vector.tensor_scalar(tmb[:], sT[:], 1.0,
                                        fb_c[:, j:j + 1], ALU.add, ALU.mult)
                nc.vector.tensor_add(b_col[:, j, :], tmb[:], hT[:])

        # ---- batch-phase pools ----
        xtp = ctx.enter_context(tc.tile_pool(name="xt", bufs=1))
        xntp = ctx.enter_context(tc.tile_pool(name="xnt", bufs=1))
        htp = ctx.enter_context(tc.tile_pool(name="ht", bufs=1))
        sqp = ctx.enter_context(tc.tile_pool(name="sq", bufs=2))
        rowp = ctx.enter_context(tc.tile_pool(name="rows", bufs=1))
        bcp = ctx.enter_context(tc.tile_pool(name="bc", bufs=1))
        bcyp = ctx.enter_context(tc.tile_pool(name="bcy", bufs=2))
        xfp = ctx.enter_context(tc.tile_pool(name="xf", bufs=1))
        kvp = ctx.enter_context(tc.tile_pool(name="kv", bufs=1))
        qp = ctx.enter_context(tc.tile_pool(name="q", bufs=1))
        ytp = ctx.enter_context(tc.tile_pool(name="yt", bufs=1))
        rbcp = ctx.enter_context(tc.tile_pool(name="rbc", bufs=2))
        tmpp = ctx.enter_context(tc.tile_pool(name="tmp", bufs=2))
        resp = ctx.enter_context(tc.tile_pool(name="res", bufs=2))
        outp = ctx.enter_context(tc.tile_pool(name="o", bufs=2))
        psq = ctx.enter_context(tc.tile_pool(name="psq", bufs=2, space=bass.MemorySpace.PSUM))
        psmid = ctx.enter_context(tc.tile_pool(name="psmid", bufs=2, space=bass.MemorySpace.PSUM))
        psst = ctx.enter_context(tc.tile_pool(name="psst", bufs=1, space=bass.MemorySpace.PSUM))
        psa = ctx.enter_context(tc.tile_pool(name="psa", bufs=1, space=bass.MemorySpace.PSUM))
        pso = ctx.enter_context(tc.tile_pool(name="pso", bufs=1, space=bass.MemorySpace.PSUM))

        inv_d = 1.0 / D
        inv_td = 1.0 / TD

        for b in range(BPC):
            # ========== x path: transpose, stats, normalize ==========
            xT = xtp.tile([128, KD, T], BF16, tag="xT")
            for j in range(KD):
                nc.sync.dma_start_transpose(xT[:, j, :],
                                            d_xbf[b, :, j * 128:(j + 1) * 128])
            rstd_xb = rowp.tile([1, T], BF16, tag="rstd_xb")
            nmr_xb = rowp.tile([1, T], BF16, tag="nmr_xb")
            xnT = xntp.tile([128, KD, T], BF16, tag="xnT")
            for chn in range(NCH):
                cs = slice(chn * 512, (chn + 1) * 512)
                s1x = psst.tile([1, 512], F32, tag="s1")
                s2x = psst.tile([1, 512], F32, tag="s2")
                for j in range(KD):
                    sq = sqp.tile([128, 512], BF16, tag="sqx")
                    nc.gpsimd.tensor_mul(sq[:], xT[:, j, cs], xT[:, j, cs])
                    nc.tensor.matmul(s1x[:], ones_c[:], xT[:, j, cs],
                                     start=(j == 0), stop=(j == KD - 1))
                    nc.tensor.matmul(s2x[:], ones_c[:], sq[:],
                                     start=(j == 0), stop=(j == KD - 1))
                # rows: mu=s1/D; u=s2-D*mu^2; rstd=arsqrt(u/D+eps)
                mu_x = rowp.tile([1, 512], F32, tag=f"r1c{chn}")
                nc.vector.tensor_scalar_mul(mu_x[:], s1x[:], inv_d)
                msq_x = rowp.tile([1, 512], F32, tag="r2")
                nc.vector.tensor_mul(msq_x[:], mu_x[:], mu_x[:])
                u_x = rowp.tile([1, 512], F32, tag=f"r3c{chn}")
                nc.vector.scalar_tensor_tensor(u_x[:], msq_x[:], -float(D), s2x[:],
                                               op0=ALU.mult, op1=ALU.add)
                nc.scalar.activation(rstd_xb[0:1, cs], u_x[:], AF.Abs_reciprocal_sqrt,
                                     bias=eps_c[0:1, :], scale=inv_d)
                nc.vector.scalar_tensor_tensor(nmr_xb[0:1, cs], mu_x[:], -1.0,
                                               rstd_xb[0:1, cs],
                                               op0=ALU.mult, op1=ALU.mult)
                rstd_bc = bcp.tile([128, 512], BF16, tag="rstd_bc")
                nc.gpsimd.partition_broadcast(rstd_bc[:], rstd_xb[0:1, cs],
                                              channels=128)
                nmr_bc = bcp.tile([128, 512], BF16, tag="nmr_bc")
                nc.gpsimd.partition_broadcast(nmr_bc[:], nmr_xb[0:1, cs],
                                              channels=128)
                for j in range(KD):
                    t1 = tmpp.tile([128, 512], BF16, tag="t1y")
                    nc.gpsimd.tensor_mul(t1[:], xT[:, j, cs], rstd_bc[:])
                    nc.gpsimd.tensor_add(xnT[:, j, cs], t1[:], nmr_bc[:])

            # ========== xf path ==========
            xfT = xfp.tile([128, KTD, N], BF16, tag="xfT")
            for kt in range(KTD):
                nc.sync.dma_start_transpose(xfT[:, kt, :],
                                            d_xfbf[b, :, kt * 128:(kt + 1) * 128])
            s1f = psst.tile([1, N], F32, tag="s1")
            s2f = psst.tile([1, N], F32, tag="s2")
            for kt in range(KTD):
                sqf = sqp.tile([128, N], BF16, tag="sqf")
                nc.vector.tensor_mul(sqf[:], xfT[:, kt, :], xfT[:, kt, :])
                nc.tensor.matmul(s1f[:], ones_c[:], xfT[:, kt, :],
                                 start=(kt == 0), stop=(kt == KTD - 1))
                nc.tensor.matmul(s2f[:], ones_c[:], sqf[:],
                                 start=(kt == 0), stop=(kt == KTD - 1))
            mu_f = rowp.tile([1, N], F32, tag="r1c0")
            nc.vector.tensor_scalar_mul(mu_f[:], s1f[:], inv_td)
            msq_f = rowp.tile([1, N], F32, tag="r2")
            nc.vector.tensor_mul(msq_f[:], mu_f[:], mu_f[:])
            u_f = rowp.tile([1, N], F32, tag="r3c0")
            nc.vector.scalar_tensor_tensor(u_f[:], msq_f[:], -float(TD), s2f[:],
                                           op0=ALU.mult, op1=ALU.add)
            rstd_fb = rowp.tile([1, N], BF16, tag="rstd_fb")
            nc.scalar.activation(rstd_fb[:], u_f[:], AF.Abs_reciprocal_sqrt,
                                 bias=eps_c[0:1, :], scale=inv_td)
            nmr_fb = rowp.tile([1, N], BF16, tag="nmr_fb")
            nc.vector.scalar_tensor_tensor(nmr_fb[:], mu_f[:], -1.0, rstd_fb[:],
                                           op0=ALU.mult, op1=ALU.mult)
            rstdf_bc = bcp.tile([128, N], BF16, tag="rstdf_bc")
            nc.gpsimd.partition_broadcast(rstdf_bc[:], rstd_fb[:], channels=128)
            nmrf_bc = bcp.tile([128, N], BF16, tag="nmrf_bc")
            nc.gpsimd.partition_broadcast(nmrf_bc[:], nmr_fb[:], channels=128)
            xfnT = xfp.tile([128, KTD, N], BF16, tag="xfnT")
            for kt in range(KTD):
                t1f = tmpp.tile([128, N], BF16, tag="t1y")
                nc.gpsimd.tensor_mul(t1f[:], xfT[:, kt, :], rstdf_bc[:])
                nc.gpsimd.tensor_add(xfnT[:, kt, :], t1f[:], nmrf_bc[:])

            # ---- K and V ----
            exp_k = kvp.tile([128, NT, D], BF16, tag="expk")
            v_sb = kvp.tile([128, NT, D], BF16, tag="vsb")
            for nt in range(NT):
                ns = slice(nt * 128, (nt + 1) * 128)
                for chn in range(NCH):
                    cs = slice(chn * 512, (chn + 1) * 512)
                    pk = psq.tile([128, 512], F32, tag="ps")
                    for kt in range(KTD):
                        nc.tensor.matmul(pk[:], xfnT[:, kt, ns], wk_sb[:, kt, cs],
                                         start=(kt == 0), stop=False)
                    nc.tensor.matmul(pk[:], ones_r[0:1, 0:128], bke_r[0:1, cs],
                                     start=False, stop=True)
                    nc.scalar.activation(exp_k[:, nt, cs], pk[:], AF.Exp)
                    pv = psq.tile([128, 512], F32, tag="ps")
                    for kt in range(KTD):
                        nc.tensor.matmul(pv[:], xfnT[:, kt, ns], wv_sb[:, kt, cs],
                                         start=(kt == 0), stop=False)
                    nc.tensor.matmul(pv[:], ones_r[0:1, 0:128], bve_r[0:1, cs],
                                     start=False, stop=True)
                    nc.vector.tensor_copy(v_sb[:, nt, cs], pv[:])

            # ---- S_k and attn ----
            pks = psa.tile([128, KD], F32, tag="skattn")
            for j in range(KD):
                for nt in range(NT):
                    nc.tensor.matmul(pks[:, j:j + 1],
                                     exp_k[:, nt, j * 128:(j + 1) * 128],
                                     ones_c[:], start=(nt == 0), stop=(nt == 1))
            r_k = rowp.tile([128, KD], F32, tag="rk")
            nc.vector.reciprocal(r_k[:], pks[:])

            patt = psa.tile([128, 512], F32, tag="skattn")
            for h in range(H):
                rp = slice((h % 2) * 64, (h % 2) * 64 + 64)
                cp = slice((h // 2) * 64, (h // 2) * 64 + 64)
                hs = slice(h * 64, (h + 1) * 64)
                for nt in range(NT):
                    nc.tensor.matmul(patt[rp, cp], exp_k[:, nt, hs],
                                     v_sb[:, nt, hs],
                                     start=(nt == 0), stop=(nt == 1))
            # block-diagonal per head pair: [0:64,0:64]=head 2j, [64:,64:]=head 2j+1
            attn_s = kvp.tile([128, KD, 128], BF16, tag="attns")
            nc.vector.memset(attn_s[:], 0.0)
            for j in range(KD):
                nc.vector.tensor_scalar_mul(attn_s[0:64, j, 0:64],
                                            patt[0:64, j * 64:(j + 1) * 64],
                                            r_k[0:64, j:j + 1])
                nc.vector.tensor_scalar_mul(attn_s[64:128, j, 64:128],
                                            patt[64:128, j * 64:(j + 1) * 64],
                                            r_k[64:128, j:j + 1])

            # ========== middle section, phase-grouped across t-chunks ==========
            exp_qT = qp.tile([128, KD, T], BF16, tag="expq")
            yT = ytp.tile([128, KD, T], BF16, tag="yT")
            hT = htp.tile([128, KD, T], BF16, tag="hT")
            # Q projection -> exp, transposed (both chunks; Exp ops adjacent)
            for ch2 in range(NCH):
                ts_ = slice(ch2 * 512, (ch2 + 1) * 512)
                for j in range(KD):
                    js = slice(j * 128, (j + 1) * 128)
                    pq = psq.tile([128, 512], F32, tag="ps")
                    for kt in range(KD):
                        nc.tensor.matmul(pq[:], wq_sb[:, kt, js],
                                         xnT[:, kt, ts_],
                                         start=(kt == 0), stop=(kt == KD - 1))
                    nc.scalar.activation(exp_qT[:, j, ts_], pq[:], AF.Exp,
                                         bias=bqc[:, j:j + 1])
            # softmax denominators + y + stats (both chunks)
            stat_ps = []
            for ch2 in range(NCH):
                ts_ = slice(ch2 * 512, (ch2 + 1) * 512)
                s_all = psq.tile([H, 512], F32, tag="ps")
                for j in range(KD):
                    nc.tensor.matmul(s_all[:], sel16[:, j, :], exp_qT[:, j, ts_],
                                     start=(j == 0), stop=(j == KD - 1))
                rs_all = rbcp.tile([H, 512], BF16, tag="rsall")
                with nc.allow_low_precision(reason="softmax recip in bf16 is fine"):
                    nc.vector.reciprocal(rs_all[:], s_all[:])
                s1y = psst.tile([1, 512], F32, tag="s1")
                s2y = psst.tile([1, 512], F32, tag="s2")
                for j in range(KD):
                    pbc = psmid.tile([128, 512], F32, tag="pm")
                    nc.tensor.matmul(pbc[:], pick[:, j, :], rs_all[:],
                                     start=True, stop=True)
                    bcs = rbcp.tile([128, 512], BF16, tag="bcs")
                    nc.vector.tensor_copy(bcs[:], pbc[:])
                    py = psmid.tile([128, 512], F32, tag="pm")
                    nc.tensor.matmul(py[:], attn_s[:, j, :], exp_qT[:, j, ts_],
                                     start=True, stop=True)
                    nc.vector.tensor_mul(yT[:, j, ts_], py[:], bcs[:])
                    sqy = sqp.tile([128, 512], BF16, tag="sqy")
                    nc.gpsimd.tensor_mul(sqy[:], yT[:, j, ts_], yT[:, j, ts_])
                    nc.tensor.matmul(s1y[:], ones_c[:], yT[:, j, ts_],
                                     start=(j == 0), stop=(j == KD - 1))
                    nc.tensor.matmul(s2y[:], ones_c[:], sqy[:],
                                     start=(j == 0), stop=(j == KD - 1))
                # drain stat psums now (DVE only): mu and u = D*var
                mu_y = rowp.tile([1, 512], F32, tag=f"r1c{ch2}")
                nc.vector.tensor_scalar_mul(mu_y[:], s1y[:], inv_d)
                msq_y = rowp.tile([1, 512], F32, tag="r2")
                nc.vector.tensor_mul(msq_y[:], mu_y[:], mu_y[:])
                u_y = rowp.tile([1, 512], F32, tag=f"r3c{ch2}")
                nc.vector.scalar_tensor_tensor(u_y[:], msq_y[:], -float(D), s2y[:],
                                               op0=ALU.mult, op1=ALU.add)
                stat_ps.append((mu_y, u_y))
            # y LN rows + broadcasts (both chunks; arsqrt ops adjacent)
            ybcs = []
            for ch2 in range(NCH):
                mu_y, u_y = stat_ps[ch2]
                rstd_yb = rowp.tile([1, 512], BF16, tag="rstd_yb")
                nc.scalar.activation(rstd_yb[:], u_y[:], AF.Abs_reciprocal_sqrt,
                                     bias=eps_c[0:1, :], scale=inv_d)
                nmr_yb = rowp.tile([1, 512], BF16, tag="nmr_yb")
                nc.vector.scalar_tensor_tensor(nmr_yb[:], mu_y[:], -1.0, rstd_yb[:],
                                               op0=ALU.mult, op1=ALU.mult)
                rstdy_bc = bcyp.tile([128, 512], BF16, tag="rstdy_bc")
                nc.gpsimd.partition_broadcast(rstdy_bc[:], rstd_yb[:], channels=128)
                nmry_bc = bcyp.tile([128, 512], BF16, tag="nmry_bc")
                nc.gpsimd.partition_broadcast(nmry_bc[:], nmr_yb[:], channels=128)
                ybcs.append((rstdy_bc, nmry_bc))
            # apply LN + FiLM + silu (both chunks; Silu ops adjacent)
            for ch2 in range(NCH):
                ts_ = slice(ch2 * 512, (ch2 + 1) * 512)
                rstdy_bc, nmry_bc = ybcs[ch2]
                for j in range(KD):
                    t1 = tmpp.tile([128, 512], BF16, tag="t1y")
                    nc.gpsimd.tensor_mul(t1[:], yT[:, j, ts_], rstdy_bc[:])
                    t2 = tmpp.tile([128, 512], BF16, tag="t2y")
                    nc.gpsimd.tensor_add(t2[:], t1[:], nmry_bc[:])
                    nc.scalar.activation(hT[:, j, ts_], t2[:], AF.Silu,
                                         bias=b_col[:, j, b:b + 1],
                                         scale=a_col[:, j, b:b + 1])
            for ch2 in range(NCH):
                # out projection + residual for the 4 t-tiles of this chunk
                for tti in range(4):
                    ti = ch2 * 4 + tti
                    trs = slice(ti * 128, (ti + 1) * 128)
                    for chn in range(NCH):
                        cs = slice(chn * 512, (chn + 1) * 512)
                        xr = resp.tile([128, 512], F32, tag="xr")
                        nc.scalar.dma_start(xr[:], d_xf32[b, trs, cs])
                        po = pso.tile([128, 512], F32, tag="po")
                        for j in range(KD):
                            nc.tensor.matmul(po[:], hT[:, j, trs], wo_sb[:, j, cs],
                                             start=(j == 0), stop=False)
                        nc.tensor.matmul(po[:], ones_r[0:1, 0:128], outb_r[0:1, cs],
                                         start=False, stop=True)
                        o_sb = outp.tile([128, 512], F32, tag="osb")
                        nc.vector.tensor_add(o_sb[:], po[:], xr[:])
                        nc.gpsimd.dma_start(d_out[b, trs, cs], o_sb[:])

    nc.compile()
    return nc


def _get_program():
    global _PROGRAM
    if _PROGRAM is None:
        _PROGRAM = _build_program()
    return _PROGRAM


def _prep_inputs(inputs):
    f = lambda k: np.asarray(inputs[k], np.float32)
    x, xf, emb = f("x"), f("xf"), f("emb")
    norm_g, norm_b = f("norm_g"), f("norm_b")
    tnorm_g, tnorm_b = f("tnorm_g"), f("tnorm_b")
    Wq, bq, Wk, bk, Wv, bv = f("Wq"), f("bq"), f("Wk"), f("bk"), f("Wv"), f("bv")
    emb_W, emb_b = f("emb_W"), f("emb_b")
    fg, fb = f("fnorm_g"), f("fnorm_b")
    out_W, out_b = f("out_W"), f("out_b")

    wq_e = norm_g[:, None] * Wq
    wk_e = tnorm_g[:, None] * Wk
    wv_e = tnorm_g[:, None] * Wv
    bq_eff = bq + norm_b @ Wq          # [D]
    sel16 = np.zeros((128, KD, H), np.float32)
    pick = np.zeros((H, KD, 128), np.float32)
    for j in range(KD):
        sel16[0:64, j, 2 * j] = 1.0
        sel16[64:128, j, 2 * j + 1] = 1.0
        pick[2 * j, j, 0:64] = 1.0
        pick[2 * j + 1, j, 64:128] = 1.0
    shared = {
        "wq": wq_e.astype(NBF), "wk": wk_e.astype(NBF), "wv": wv_e.astype(NBF),
        "wo": out_W.astype(NBF), "wemb": emb_W.astype(NBF),
        "bqc": np.ascontiguousarray(bq_eff.reshape(KD, 128).T),
        "bke": (bk + tnorm_b @ Wk).astype(NBF),
        "bve": (bv + tnorm_b @ Wv).astype(NBF),
        "outb": out_b.astype(NBF), "embb": emb_b.astype(NBF),
        "fgc": np.ascontiguousarray(fg.reshape(KD, 128).T),
        "fbc": np.ascontiguousarray(fb.reshape(KD, 128).T),
        "sel16": sel16.astype(NBF), "pick": pick.astype(NBF),
    }
    xbf = x.astype(NBF)
    xfbf = xf.astype(NBF)
    in_maps = []
    for i in range(NCORES):
        s = slice(i * BPC, (i + 1) * BPC)
        m = dict(shared)
        m["xbf"] = xbf[s]
        m["xf32"] = x[s]
        m["xfbf"] = xfbf[s]
        m["emb"] = emb[s]
        in_maps.append(m)
    return in_maps


def run(inputs, trace=False):
    nc = _get_program()
    in_maps = _prep_inputs(inputs)
    res = run_bass_kernel_spmd(nc, in_maps, core_ids=list(range(NCORES)),
                               trace=trace)
    out = np.concatenate([res.results[i]["out"] for i in range(NCORES)], axis=0)
    return out, res


def kernel(**inputs):
    out, _ = run(inputs, trace=False)
    return out


# revision 22
# speedup vs baseline: 1.5811x; 1.5811x over previous
"""Trainium2 Bass kernel for nn_CrossAttention (linear cross-attention block).

Computation (per batch b):
  xn  = LN(x[b]; norm_g, norm_b)                 [T, D]
  xfn = LN(xf[b]; tnorm_g, tnorm_b)              [N, TD]
  q   = softmax_c((xn @ Wq + bq).reshape(T,H,C))
  k   = softmax_n((xfn @ Wk + bk).reshape(N,H,C))
  v   = (xfn @ Wv + bv).reshape(N,H,C)
  attn= einsum('nhc,nhd->hcd', k, v); y = einsum('thc,hcd->thd', q, attn)
  e   = silu(emb) @ emb_W + emb_b; scale, shift = split(e)
  h   = LN(y; fnorm_g, fnorm_b) * (1+scale) + shift
  out = x + silu(h) @ out_W + out_b

Sharding: pure data-parallel over batch B=32 across 8 NeuronCores (4 each).

Device strategy: the whole middle section (Q-softmax -> y -> LN -> FiLM ->
silu) runs in TRANSPOSED layout [d-part, t-free], so no per-tile SBUF
transposes are needed:
  - qT comes straight out of the Q projection (lhsT=Wq, rhs=xnT);
  - the q-softmax denominator is broadcast with a block-diagonal ones
    matmul (one [128,128] matmul per tile) + DVE reciprocal;
  - y is produced transposed by using the block-diagonal attn tile as lhsT;
  - LN stats are partition-sums via ones-column matmuls; rstd via the
    scalar engine's Abs_reciprocal_sqrt; mean/rstd rows broadcast across
    partitions by gpsimd;
  - FiLM scale/bias are per-partition columns in this layout, fused into a
    single Silu activation (scale=A, bias=B);
  - silu(h) lands transposed = exactly the lhsT the out-projection needs.
x enters only via 8 big DMA transposes per batch (raw bf16); LN of x/xf is
applied in transposed space (gain/bias pre-folded into W/proj biases on the
host).
"""

from contextlib import ExitStack

import numpy as np
import ml_dtypes

import concourse.bass as bass
import concourse.mybir as mybir
import concourse.tile as tile
from concourse import bacc
from concourse.bass_utils import run_bass_kernel_spmd
from concourse.masks import make_identity

# problem shapes (hardcoded per contract)
B, T, N, D, TD, H, C, TE = 32, 1024, 256, 1024, 768, 16, 64, 2048
D2 = 2 * D
EPS = 1e-5
NCORES = 8
BPC = B // NCORES           # batches per core
TI = T // 128               # 8 t-tiles
KD = D // 128               # 8 k-tiles over D
KTD = TD // 128             # 6 k-tiles over TD
KTE = TE // 128             # 16 k-tiles over TE
NT = N // 128               # 2 n-tiles
NCH = D // 512              # 2 free 512-chunks over D

F32 = mybir.dt.float32
BF16 = mybir.dt.bfloat16
AF = mybir.ActivationFunctionType
ALU = mybir.AluOpType
NBF = ml_dtypes.bfloat16

_PROGRAM = None  # cached (nc) build


def _build_program():
    nc = bacc.Bacc("TRN2", target_bir_lowering=False, debug=False,
                   num_devices=NCORES)

    # ---- DRAM I/O ----
    d_xbf = nc.dram_tensor("xbf", [BPC, T, D], BF16, kind="ExternalInput")
    d_xf32 = nc.dram_tensor("xf32", [BPC, T, D], F32, kind="ExternalInput")
    d_xfbf = nc.dram_tensor("xfbf", [BPC, N, TD], BF16, kind="ExternalInput")
    d_emb = nc.dram_tensor("emb", [BPC, TE], F32, kind="ExternalInput")
    d_wq = nc.dram_tensor("wq", [D, D], BF16, kind="ExternalInput")
    d_wk = nc.dram_tensor("wk", [TD, D], BF16, kind="ExternalInput")
    d_wv = nc.dram_tensor("wv", [TD, D], BF16, kind="ExternalInput")
    d_wo = nc.dram_tensor("wo", [D, D], BF16, kind="ExternalInput")
    d_wemb = nc.dram_tensor("wemb", [TE, D2], BF16, kind="ExternalInput")
    d_bqc = nc.dram_tensor("bqc", [128, KD], F32, kind="ExternalInput")
    d_bke = nc.dram_tensor("bke", [D], BF16, kind="ExternalInput")
    d_bve = nc.dram_tensor("bve", [D], BF16, kind="ExternalInput")
    d_embb = nc.dram_tensor("embb", [D2], BF16, kind="ExternalInput")
    d_fgc = nc.dram_tensor("fgc", [128, KD], F32, kind="ExternalInput")
    d_fbc = nc.dram_tensor("fbc", [128, KD], F32, kind="ExternalInput")
    d_sel16 = nc.dram_tensor("sel16", [128, KD, H], BF16, kind="ExternalInput")
    d_pick = nc.dram_tensor("pick", [H, KD, 128], BF16, kind="ExternalInput")
    d_out = nc.dram_tensor("out", [BPC, T, D], F32, kind="ExternalOutput")

    with tile.TileContext(nc) as tc, ExitStack() as ctx:
        wpool = ctx.enter_context(tc.tile_pool(name="weights", bufs=1))
        cpool = ctx.enter_context(tc.tile_pool(name="consts", bufs=1))

        # ---- persistent weights ----
        wq_sb = wpool.tile([128, KD, D], BF16)
        nc.scalar.dma_start(wq_sb[:], d_wq[:].rearrange("(i p) m -> p i m", p=128))
        wk_sb = wpool.tile([128, KTD, D], BF16)
        nc.scalar.dma_start(wk_sb[:], d_wk[:].rearrange("(i p) m -> p i m", p=128))
        wv_sb = wpool.tile([128, KTD, D], BF16)
        nc.scalar.dma_start(wv_sb[:], d_wv[:].rearrange("(i p) m -> p i m", p=128))
        wo_sb = wpool.tile([128, KD, D], BF16)
        nc.scalar.dma_start(wo_sb[:], d_wo[:].rearrange("(i p) m -> p i m", p=128))

        # ---- constants ----
        bqc = cpool.tile([128, KD], F32)
        nc.scalar.dma_start(bqc[:], d_bqc[:])
        sel16 = cpool.tile([128, KD, H], BF16)
        nc.scalar.dma_start(sel16[:], d_sel16[:])
        pick = cpool.tile([H, KD, 128], BF16)
        nc.scalar.dma_start(pick[:], d_pick[:])
        bke_r = cpool.tile([1, D], BF16)
        nc.scalar.dma_start(bke_r[:], d_bke[None, :])
        bve_r = cpool.tile([1, D], BF16)
        nc.scalar.dma_start(bve_r[:], d_bve[None, :])
        ones_r = cpool.tile([1, 128], BF16)
        nc.vector.memset(ones_r[:], 1.0)
        ones_c = cpool.tile([128, 1], BF16)
        nc.vector.memset(ones_c[:], 1.0)
        eps_c = cpool.tile([128, 1], F32)
        nc.vector.memset(eps_c[:], EPS)
        ident = cpool.tile([128, 128], BF16)
        make_identity(nc, ident[:])
        identf = cpool.tile([4, 4], F32)
        make_identity(nc, identf[:])
        identf128 = cpool.tile([128, 128], F32)
        make_identity(nc, identf128[:])

        a_col = cpool.tile([128, KD, BPC], F32)   # FiLM A columns
        b_col = cpool.tile([128, KD, BPC], F32)   # FiLM B columns

        # ---- emb / FiLM phase (all BPC batches at once) ----
        with tc.tile_pool(name="wemb", bufs=1) as wep, \
             tc.tile_pool(name="etmp", bufs=1) as ep, \
             tc.tile_pool(name="pse", bufs=2, space=bass.MemorySpace.PSUM) as pse:
            wemb_sb = wep.tile([128, KTE, D2], BF16)
            embb_r = ep.tile([1, D2], BF16)
            nc.scalar.dma_start(embb_r[:], d_embb[None, :])
            fg_c = ep.tile([128, KD], F32)
            nc.scalar.dma_start(fg_c[:], d_fgc[:])
            fb_c = ep.tile([128, KD], F32)
            nc.scalar.dma_start(fb_c[:], d_fbc[:])
            nc.scalar.dma_start(wemb_sb[:],
                              d_wemb[:].rearrange("(i p) m -> p i m", p=128))
            emb_sb = ep.tile([BPC, TE], F32)
            nc.scalar.dma_start(emb_sb[:], d_emb[:])
            semb = ep.tile([BPC, TE], BF16)
            nc.scalar.activation(semb[:], emb_sb[:], AF.Silu)
            embT = ep.tile([128, KTE, BPC], BF16)
            for c in range(KTE):
                pst = pse.tile([128, BPC], BF16, tag="pst")
                nc.tensor.transpose(pst[:], semb[:, c * 128:(c + 1) * 128],
                                    ident[0:BPC, 0:BPC])
                nc.vector.tensor_copy(embT[:, c, :], pst[:])
            e_sb = ep.tile([BPC, D2], F32)
            for chn in range(D2 // 512):
                pe = pse.tile([BPC, 512], F32, tag="pe")
                for kt in range(KTE):
                    nc.tensor.matmul(pe[:], embT[:, kt, :],
                                     wemb_sb[:, kt, chn * 512:(chn + 1) * 512],
                                     start=(kt == 0), stop=False)
                nc.tensor.matmul(pe[:], ones_r[0:1, 0:BPC],
                                 embb_r[0:1, chn * 512:(chn + 1) * 512],
                                 start=False, stop=True)
                nc.vector.tensor_copy(e_sb[:, chn * 512:(chn + 1) * 512], pe[:])
            # transpose scale/shift to columns, build A/B FiLM columns
            for j in range(KD):
                js = slice(j * 128, (j + 1) * 128)
                pts = pse.tile([128, BPC], F32, tag="pts")
                nc.tensor.transpose(pts[:], e_sb[0:BPC, js], identf[:])
                sT = ep.tile([128, BPC], F32, tag="sT")
                nc.vector.tensor_copy(sT[:], pts[:])
                pth = pse.tile([128, BPC], F32, tag="pts")
                nc.tensor.transpose(pth[:], e_sb[0:BPC, D + j * 128:D + (j + 1) * 128],
                                    identf[:])
                hT = ep.tile([128, BPC], F32, tag="hT")
                nc.vector.tensor_copy(hT[:], pth[:])
                nc.vector.tensor_scalar(a_col[:, j, :], sT[:], 1.0,
                                        fg_c[:, j:j + 1], ALU.add, ALU.mult)
                tmb = ep.tile([128, BPC], F32, tag="tmb")
                nc.vector.tensor_scalar(tmb[:], sT[:], 1.0,
                                        fb_c[:, j:j + 1], ALU.add, ALU.mult)
                nc.vector.tensor_add(b_col[:, j, :], tmb[:], hT[:])

        # ---- batch-phase pools ----
        xtp = ctx.enter_context(tc.tile_pool(name="xt", bufs=1))
        xntp = ctx.enter_context(tc.tile_pool(name="xnt", bufs=1))
        htp = ctx.enter_context(tc.tile_pool(name="ht", bufs=1))
        sqp = ctx.enter_context(tc.tile_pool(name="sq", bufs=2))
        rowp = ctx.enter_context(tc.tile_pool(name="rows", bufs=1))
        bcp = ctx.enter_context(tc.tile_pool(name="bc", bufs=1))
        bcyp = ctx.enter_context(tc.tile_pool(name="bcy", bufs=2))
        xfp = ctx.enter_context(tc.tile_pool(name="xf", bufs=1))
        kvp = ctx.enter_context(tc.tile_pool(name="kv", bufs=1))
        qp = ctx.enter_context(tc.tile_pool(name="q", bufs=1))
        ytp = ctx.enter_context(tc.tile_pool(name="yt", bufs=1))
        rbcp = ctx.enter_context(tc.tile_pool(name="rbc", bufs=2))
        tmpp = ctx.enter_context(tc.tile_pool(name="tmp", bufs=2))
        resp = ctx.enter_context(tc.tile_pool(name="res", bufs=2))
        outp = ctx.enter_context(tc.tile_pool(name="o", bufs=2))
        psq = ctx.enter_context(tc.tile_pool(name="psq", bufs=2, space=bass.MemorySpace.PSUM))
        psmid = ctx.enter_context(tc.tile_pool(name="psmid", bufs=2, space=bass.MemorySpace.PSUM))
        psst = ctx.enter_context(tc.tile_pool(name="psst", bufs=1, space=bass.MemorySpace.PSUM))
        psa = ctx.enter_context(tc.tile_pool(name="psa", bufs=1, space=bass.MemorySpace.PSUM))
        pso = ctx.enter_context(tc.tile_pool(name="pso", bufs=1, space=bass.MemorySpace.PSUM))

        inv_d = 1.0 / D
        inv_td = 1.0 / TD

        for b in range(BPC):
            # ========== x path: transpose, stats, normalize ==========
            xT = xtp.tile([128, KD, T], BF16, tag="xT")
            for j in range(KD):
                nc.sync.dma_start_transpose(xT[:, j, :],
                                            d_xbf[b, :, j * 128:(j + 1) * 128])
            rstd_xb = rowp.tile([1, T], BF16, tag="rstd_xb")
            nmr_xb = rowp.tile([1, T], BF16, tag="nmr_xb")
            xnT = xntp.tile([128, KD, T], BF16, tag="xnT")
            for ti in range(TI):
                trs = slice(ti * 128, (ti + 1) * 128)
                xbn = sqp.tile([128, D], BF16, tag="xbn")
                nc.gpsimd.dma_start(xbn[:], d_xbf[b, trs, :])
                st = sqp.tile([128, 2, 6], F32, tag="stx")
                nc.vector.bn_stats(st[:, 0, :], xbn[:, 0:512])
                nc.vector.bn_stats(st[:, 1, :], xbn[:, 512:1024])
                mv = sqp.tile([128, 2], F32, tag="mvx")
                nc.vector.bn_aggr(mv[:], st[:])
                rstd_c = sqp.tile([128, 1], F32, tag="rstdc")
                nc.scalar.activation(rstd_c[:], mv[:, 1:2], AF.Abs_reciprocal_sqrt,
                                     bias=eps_c[:])
                nmr_c = sqp.tile([128, 1], F32, tag="nmrc")
                nc.vector.scalar_tensor_tensor(nmr_c[:], mv[:, 0:1], -1.0,
                                               rstd_c[:], op0=ALU.mult, op1=ALU.mult)
                ptr1 = psst.tile([1, 128], F32, tag="s1")
                nc.tensor.transpose(ptr1[:], rstd_c[:], identf128[:])
                nc.vector.tensor_copy(rstd_xb[0:1, trs], ptr1[:])
                ptr2 = psst.tile([1, 128], F32, tag="s2")
                nc.tensor.transpose(ptr2[:], nmr_c[:], identf128[:])
                nc.vector.tensor_copy(nmr_xb[0:1, trs], ptr2[:])
            for chn in range(NCH):
                cs = slice(chn * 512, (chn + 1) * 512)
                rstd_bc = bcp.tile([128, 512], BF16, tag="rstd_bc")
                nc.gpsimd.partition_broadcast(rstd_bc[:], rstd_xb[0:1, cs],
                                              channels=128)
                nmr_bc = bcp.tile([128, 512], BF16, tag="nmr_bc")
                nc.gpsimd.partition_broadcast(nmr_bc[:], nmr_xb[0:1, cs],
                                              channels=128)
                for j in range(KD):
                    t1 = tmpp.tile([128, 512], BF16, tag="t1y")
                    nc.vector.tensor_mul(t1[:], xT[:, j, cs], rstd_bc[:])
                    nc.vector.tensor_add(xnT[:, j, cs], t1[:], nmr_bc[:])

            # ========== xf path ==========
            xfT = xfp.tile([128, KTD, N], BF16, tag="xfT")
            for kt in range(KTD):
                nc.sync.dma_start_transpose(xfT[:, kt, :],
                                            d_xfbf[b, :, kt * 128:(kt + 1) * 128])
            s1f = psst.tile([1, N], F32, tag="s1")
            s2f = psst.tile([1, N], F32, tag="s2")
            for kt in range(KTD):
                sqf = sqp.tile([128, N], BF16, tag="sqf")
                nc.vector.tensor_mul(sqf[:], xfT[:, kt, :], xfT[:, kt, :])
                nc.tensor.matmul(s1f[:], ones_c[:], xfT[:, kt, :],
                                 start=(kt == 0), stop=(kt == KTD - 1))
                nc.tensor.matmul(s2f[:], ones_c[:], sqf[:],
                                 start=(kt == 0), stop=(kt == KTD - 1))
            mu_f = rowp.tile([1, N], F32, tag="r1c0")
            nc.vector.tensor_scalar_mul(mu_f[:], s1f[:], inv_td)
            msq_f = rowp.tile([1, N], F32, tag="r2")
            nc.vector.tensor_mul(msq_f[:], mu_f[:], mu_f[:])
            u_f = rowp.tile([1, N], F32, tag="r3c0")
            nc.vector.scalar_tensor_tensor(u_f[:], msq_f[:], -float(TD), s2f[:],
                                           op0=ALU.mult, op1=ALU.add)
            rstd_fb = rowp.tile([1, N], BF16, tag="rstd_fb")
            nc.scalar.activation(rstd_fb[:], u_f[:], AF.Abs_reciprocal_sqrt,
                                 bias=eps_c[0:1, :], scale=inv_td)
            nmr_fb = rowp.tile([1, N], BF16, tag="nmr_fb")
            nc.vector.scalar_tensor_tensor(nmr_fb[:], mu_f[:], -1.0, rstd_fb[:],
                                           op0=ALU.mult, op1=ALU.mult)
            rstdf_bc = bcp.tile([128, N], BF16, tag="rstdf_bc")
            nc.gpsimd.partition_broadcast(rstdf_bc[:], rstd_fb[:], channels=128)
            nmrf_bc = bcp.tile([128, N], BF16, tag="nmrf_bc")
            nc.gpsimd.partition_broadcast(nmrf_bc[:], nmr_fb[:], channels=128)
            xfnT = xfp.tile([128, KTD, N], BF16, tag="xfnT")
            for kt in range(KTD):
                t1f = tmpp.tile([128, N], BF16, tag="t1y")
                nc.vector.tensor_mul(t1f[:], xfT[:, kt, :], rstdf_bc[:])
                nc.vector.tensor_add(xfnT[:, kt, :], t1f[:], nmrf_bc[:])

            # ---- K and V ----
            exp_k = kvp.tile([128, NT, D], BF16, tag="expk")
            v_sb = kvp.tile([128, NT, D], BF16, tag="vsb")
            for nt in range(NT):
                ns = slice(nt * 128, (nt + 1) * 128)
                for chn in range(NCH):
                    cs = slice(chn * 512, (chn + 1) * 512)
                    pk = psq.tile([128, 512], F32, tag="ps")
                    for kt in range(KTD):
                        nc.tensor.matmul(pk[:], xfnT[:, kt, ns], wk_sb[:, kt, cs],
                                         start=(kt == 0), stop=False)
                    nc.tensor.matmul(pk[:], ones_r[0:1, 0:128], bke_r[0:1, cs],
                                     start=False, stop=True)
                    nc.scalar.activation(exp_k[:, nt, cs], pk[:], AF.Exp)
                    pv = psq.tile([128, 512], F32, tag="ps")
                    for kt in range(KTD):
                        nc.tensor.matmul(pv[:], xfnT[:, kt, ns], wv_sb[:, kt, cs],
                                         start=(kt == 0), stop=False)
                    nc.tensor.matmul(pv[:], ones_r[0:1, 0:128], bve_r[0:1, cs],
                                     start=False, stop=True)
                    nc.vector.tensor_copy(v_sb[:, nt, cs], pv[:])

            # ---- S_k and attn ----
            pks = psa.tile([128, KD], F32, tag="skattn")
            for j in range(KD):
                for nt in range(NT):
                    nc.tensor.matmul(pks[:, j:j + 1],
                                     exp_k[:, nt, j * 128:(j + 1) * 128],
                                     ones_c[:], start=(nt == 0), stop=(nt == 1))
            r_k = rowp.tile([128, KD], F32, tag="rk")
            nc.vector.reciprocal(r_k[:], pks[:])

            patt = psa.tile([128, 512], F32, tag="skattn")
            for h in range(H):
                rp = slice((h % 2) * 64, (h % 2) * 64 + 64)
                cp = slice((h // 2) * 64, (h // 2) * 64 + 64)
                hs = slice(h * 64, (h + 1) * 64)
                for nt in range(NT):
                    nc.tensor.matmul(patt[rp, cp], exp_k[:, nt, hs],
                                     v_sb[:, nt, hs],
                                     start=(nt == 0), stop=(nt == 1))
            # block-diagonal per head pair: [0:64,0:64]=head 2j, [64:,64:]=head 2j+1
            attn_s = kvp.tile([128, KD, 128], BF16, tag="attns")
            nc.vector.memset(attn_s[:], 0.0)
            for j in range(KD):
                nc.vector.tensor_scalar_mul(attn_s[0:64, j, 0:64],
                                            patt[0:64, j * 64:(j + 1) * 64],
                                            r_k[0:64, j:j + 1])
                nc.vector.tensor_scalar_mul(attn_s[64:128, j, 64:128],
                                            patt[64:128, j * 64:(j + 1) * 64],
                                            r_k[64:128, j:j + 1])

            # ========== middle section, phase-grouped across t-chunks ==========
            exp_qT = qp.tile([128, KD, T], BF16, tag="expq")
            yT = ytp.tile([128, KD, T], BF16, tag="yT")
            hT = htp.tile([128, KD, T], BF16, tag="hT")
            # Q projection -> exp, transposed (both chunks; Exp ops adjacent)
            for ch2 in range(NCH):
                ts_ = slice(ch2 * 512, (ch2 + 1) * 512)
                for j in range(KD):
                    js = slice(j * 128, (j + 1) * 128)
                    pq = psq.tile([128, 512], F32, tag="ps")
                    for kt in range(KD):
                        nc.tensor.matmul(pq[:], wq_sb[:, kt, js],
                                         xnT[:, kt, ts_],
                                         start=(kt == 0), stop=(kt == KD - 1))
                    nc.scalar.activation(exp_qT[:, j, ts_], pq[:], AF.Exp,
                                         bias=bqc[:, j:j + 1])
            # softmax denominators + y + stats (both chunks)
            stat_ps = []
            for ch2 in range(NCH):
                ts_ = slice(ch2 * 512, (ch2 + 1) * 512)
                s_all = psq.tile([H, 512], F32, tag="ps")
                for j in range(KD):
                    nc.tensor.matmul(s_all[:], sel16[:, j, :], exp_qT[:, j, ts_],
                                     start=(j == 0), stop=(j == KD - 1))
                rs_all = rbcp.tile([H, 512], BF16, tag="rsall")
                with nc.allow_low_precision(reason="softmax recip in bf16 is fine"):
                    nc.vector.reciprocal(rs_all[:], s_all[:])
                s1y = psst.tile([1, 512], F32, tag="s1")
                s2y = psst.tile([1, 512], F32, tag="s2")
                for j in range(KD):
                    pbc = psmid.tile([128, 512], F32, tag="pm")
                    nc.tensor.matmul(pbc[:], pick[:, j, :], rs_all[:],
                                     start=True, stop=True)
                    bcs = rbcp.tile([128, 512], BF16, tag="bcs")
                    nc.vector.tensor_copy(bcs[:], pbc[:])
                    py = psmid.tile([128, 512], F32, tag="pm")
                    nc.tensor.matmul(py[:], attn_s[:, j, :], exp_qT[:, j, ts_],
                                     start=True, stop=True)
                    nc.vector.tensor_mul(yT[:, j, ts_], py[:], bcs[:])
                    sqy = sqp.tile([128, 512], BF16, tag="sqy")
                    nc.vector.tensor_mul(sqy[:], yT[:, j, ts_], yT[:, j, ts_])
                    nc.tensor.matmul(s1y[:], ones_c[:], yT[:, j, ts_],
                                     start=(j == 0), stop=(j == KD - 1))
                    nc.tensor.matmul(s2y[:], ones_c[:], sqy[:],
                                     start=(j == 0), stop=(j == KD - 1))
                # drain stat psums now (DVE only): mu and u = D*var
                mu_y = rowp.tile([1, 512], F32, tag=f"r1c{ch2}")
                nc.vector.tensor_scalar_mul(mu_y[:], s1y[:], inv_d)
                msq_y = rowp.tile([1, 512], F32, tag="r2")
                nc.vector.tensor_mul(msq_y[:], mu_y[:], mu_y[:])
                u_y = rowp.tile([1, 512], F32, tag=f"r3c{ch2}")
                nc.vector.scalar_tensor_tensor(u_y[:], msq_y[:], -float(D), s2y[:],
                                               op0=ALU.mult, op1=ALU.add)
                stat_ps.append((mu_y, u_y))
            # y LN rows + broadcasts (both chunks; arsqrt ops adjacent)
            ybcs = []
            for ch2 in range(NCH):
                mu_y, u_y = stat_ps[ch2]
                rstd_yb = rowp.tile([1, 512], BF16, tag="rstd_yb")
                nc.scalar.activation(rstd_yb[:], u_y[:], AF.Abs_reciprocal_sqrt,
                                     bias=eps_c[0:1, :], scale=inv_d)
                nmr_yb = rowp.tile([1, 512], BF16, tag="nmr_yb")
                nc.vector.scalar_tensor_tensor(nmr_yb[:], mu_y[:], -1.0, rstd_yb[:],
                                               op0=ALU.mult, op1=ALU.mult)
                rstdy_bc = bcyp.tile([128, 512], BF16, tag="rstdy_bc")
                nc.gpsimd.partition_broadcast(rstdy_bc[:], rstd_yb[:], channels=128)
                nmry_bc = bcyp.tile([128, 512], BF16, tag="nmry_bc")
                nc.gpsimd.partition_broadcast(nmry_bc[:], nmr_yb[:], channels=128)
                ybcs.append((rstdy_bc, nmry_bc))
            # apply LN + FiLM + silu (both chunks; Silu ops adjacent)
            for ch2 in range(NCH):
                ts_ = slice(ch2 * 512, (ch2 + 1) * 512)
                rstdy_bc, nmry_bc = ybcs[ch2]
                for j in range(KD):
                    t1 = tmpp.tile([128, 512], BF16, tag="t1y")
                    nc.vector.tensor_mul(t1[:], yT[:, j, ts_], rstdy_bc[:])
                    t2 = tmpp.tile([128, 512], BF16, tag="t2y")
                    nc.vector.tensor_add(t2[:], t1[:], nmry_bc[:])
                    nc.scalar.activation(hT[:, j, ts_], t2[:], AF.Silu,
                                         bias=b_col[:, j, b:b + 1],
                                         scale=a_col[:, j, b:b + 1])
            for ch2 in range(NCH):
                # out projection + residual for the 4 t-tiles of this chunk
                for tti in range(4):
                    ti = ch2 * 4 + tti
                    trs = slice(ti * 128, (ti + 1) * 128)
                    for chn in range(NCH):
                        cs = slice(chn * 512, (chn + 1) * 512)
                        xr = resp.tile([128, 512], F32, tag="xr")
                        nc.scalar.dma_start(xr[:], d_xf32[b, trs, cs])
                        po = pso.tile([128, 512], F32, tag="po")
                        for j in range(KD):
                            nc.tensor.matmul(po[:], hT[:, j, trs], wo_sb[:, j, cs],
                                             start=(j == 0), stop=(j == KD - 1))
                        o_sb = outp.tile([128, 512], F32, tag="osb")
                        nc.vector.tensor_add(o_sb[:], po[:], xr[:])
                        nc.gpsimd.dma_start(d_out[b, trs, cs], o_sb[:])

    nc.compile()
    return nc


def _get_program():
    global _PROGRAM
    if _PROGRAM is None:
        _PROGRAM = _build_program()
    return _PROGRAM


def _prep_inputs(inputs):
    f = lambda k: np.asarray(inputs[k], np.float32)
    x, xf, emb = f("x"), f("xf"), f("emb")
    norm_g, norm_b = f("norm_g"), f("norm_b")
    tnorm_g, tnorm_b = f("tnorm_g"), f("tnorm_b")
    Wq, bq, Wk, bk, Wv, bv = f("Wq"), f("bq"), f("Wk"), f("bk"), f("Wv"), f("bv")
    emb_W, emb_b = f("emb_W"), f("emb_b")
    fg, fb = f("fnorm_g"), f("fnorm_b")
    out_W, out_b = f("out_W"), f("out_b")

    wq_e = norm_g[:, None] * Wq
    wk_e = tnorm_g[:, None] * Wk
    wv_e = tnorm_g[:, None] * Wv
    bq_eff = bq + norm_b @ Wq          # [D]
    sel16 = np.zeros((128, KD, H), np.float32)
    pick = np.zeros((H, KD, 128), np.float32)
    for j in range(KD):
        sel16[0:64, j, 2 * j] = 1.0
        sel16[64:128, j, 2 * j + 1] = 1.0
        pick[2 * j, j, 0:64] = 1.0
        pick[2 * j + 1, j, 64:128] = 1.0
    shared = {
        "wq": wq_e.astype(NBF), "wk": wk_e.astype(NBF), "wv": wv_e.astype(NBF),
        "wo": out_W.astype(NBF), "wemb": emb_W.astype(NBF),
        "bqc": np.ascontiguousarray(bq_eff.reshape(KD, 128).T),
        "bke": (bk + tnorm_b @ Wk).astype(NBF),
        "bve": (bv + tnorm_b @ Wv).astype(NBF),
        "embb": emb_b.astype(NBF),
        "fgc": np.ascontiguousarray(fg.reshape(KD, 128).T),
        "fbc": np.ascontiguousarray(fb.reshape(KD, 128).T),
        "sel16": sel16.astype(NBF), "pick": pick.astype(NBF),
    }
    xbf = x.astype(NBF)
    xfbf = xf.astype(NBF)
    xres = x + out_b[None, None, :]
    in_maps = []
    for i in range(NCORES):
        s = slice(i * BPC, (i + 1) * BPC)
        m = dict(shared)
        m["xbf"] = xbf[s]
        m["xf32"] = xres[s]
        m["xfbf"] = xfbf[s]
        m["emb"] = emb[s]
        in_maps.append(m)
    return in_maps


def run(inputs, trace=False):
    nc = _get_program()
    in_maps = _prep_inputs(inputs)
    res = run_bass_kernel_spmd(nc, in_maps, core_ids=list(range(NCORES)),
                               trace=trace)
    out = np.concatenate([res.results[i]["out"] for i in range(NCORES)], axis=0)
    return out, res


def kernel(**inputs):
    out, _ = run(inputs, trace=False)
    return out


# revision 23
# speedup vs baseline: 1.7920x; 1.1333x over previous
"""Trainium2 Bass kernel for nn_CrossAttention (linear cross-attention block).

Computation (per batch b):
  xn  = LN(x[b]; norm_g, norm_b)                 [T, D]
  xfn = LN(xf[b]; tnorm_g, tnorm_b)              [N, TD]
  q   = softmax_c((xn @ Wq + bq).reshape(T,H,C))
  k   = softmax_n((xfn @ Wk + bk).reshape(N,H,C))
  v   = (xfn @ Wv + bv).reshape(N,H,C)
  attn= einsum('nhc,nhd->hcd', k, v); y = einsum('thc,hcd->thd', q, attn)
  e   = silu(emb) @ emb_W + emb_b; scale, shift = split(e)
  h   = LN(y; fnorm_g, fnorm_b) * (1+scale) + shift
  out = x + silu(h) @ out_W + out_b

Sharding: pure data-parallel over batch B=32 across 8 NeuronCores (4 each).

Device strategy: the whole middle section (Q-softmax -> y -> LN -> FiLM ->
silu) runs in TRANSPOSED layout [d-part, t-free], so no per-tile SBUF
transposes are needed:
  - qT comes straight out of the Q projection (lhsT=Wq, rhs=xnT);
  - the q-softmax denominator is broadcast with a block-diagonal ones
    matmul (one [128,128] matmul per tile) + DVE reciprocal;
  - y is produced transposed by using the block-diagonal attn tile as lhsT;
  - LN stats are partition-sums via ones-column matmuls; rstd via the
    scalar engine's Abs_reciprocal_sqrt; mean/rstd rows broadcast across
    partitions by gpsimd;
  - FiLM scale/bias are per-partition columns in this layout, fused into a
    single Silu activation (scale=A, bias=B);
  - silu(h) lands transposed = exactly the lhsT the out-projection needs.
x enters only via 8 big DMA transposes per batch (raw bf16); LN of x/xf is
applied in transposed space (gain/bias pre-folded into W/proj biases on the
host).
"""

from contextlib import ExitStack

import numpy as np
import ml_dtypes

import concourse.bass as bass
import concourse.mybir as mybir
import concourse.tile as tile
from concourse import bacc
from concourse.bass_utils import run_bass_kernel_spmd
from concourse.masks import make_identity

# problem shapes (hardcoded per contract)
B, T, N, D, TD, H, C, TE = 32, 1024, 256, 1024, 768, 16, 64, 2048
D2 = 2 * D
EPS = 1e-5
NCORES = 8
BPC = B // NCORES           # batches per core
TI = T // 128               # 8 t-tiles
KD = D // 128               # 8 k-tiles over D
KTD = TD // 128             # 6 k-tiles over TD
KTE = TE // 128             # 16 k-tiles over TE
NT = N // 128               # 2 n-tiles
NCH = D // 512              # 2 free 512-chunks over D

F32 = mybir.dt.float32
BF16 = mybir.dt.bfloat16
AF = mybir.ActivationFunctionType
ALU = mybir.AluOpType
NBF = ml_dtypes.bfloat16

_PROGRAM = None  # cached (nc) build


def _build_program():
    nc = bacc.Bacc("TRN2", target_bir_lowering=False, debug=False,
                   num_devices=NCORES)

    # ---- DRAM I/O ----
    d_xbf = nc.dram_tensor("xbf", [BPC, T, D], BF16, kind="ExternalInput")
    d_xf32 = nc.dram_tensor("xf32", [BPC, T, D], F32, kind="ExternalInput")
    d_xfbf = nc.dram_tensor("xfbf", [BPC, N, TD], BF16, kind="ExternalInput")
    d_emb = nc.dram_tensor("emb", [BPC, TE], F32, kind="ExternalInput")
    d_wq = nc.dram_tensor("wq", [D, D], BF16, kind="ExternalInput")
    d_wk = nc.dram_tensor("wk", [TD, D], BF16, kind="ExternalInput")
    d_wv = nc.dram_tensor("wv", [TD, D], BF16, kind="ExternalInput")
    d_wo = nc.dram_tensor("wo", [D, D], BF16, kind="ExternalInput")
    d_wemb = nc.dram_tensor("wemb", [TE, D2], BF16, kind="ExternalInput")
    d_bqc = nc.dram_tensor("bqc", [128, KD], F32, kind="ExternalInput")
    d_bke = nc.dram_tensor("bke", [D], BF16, kind="ExternalInput")
    d_bve = nc.dram_tensor("bve", [D], BF16, kind="ExternalInput")
    d_embb = nc.dram_tensor("embb", [D2], BF16, kind="ExternalInput")
    d_fgc = nc.dram_tensor("fgc", [128, KD], F32, kind="ExternalInput")
    d_fbc = nc.dram_tensor("fbc", [128, KD], F32, kind="ExternalInput")
    d_sel16 = nc.dram_tensor("sel16", [128, KD, H], BF16, kind="ExternalInput")
    d_pick = nc.dram_tensor("pick", [H, KD, 128], BF16, kind="ExternalInput")
    d_out = nc.dram_tensor("out", [BPC, T, D], F32, kind="ExternalOutput")

    with tile.TileContext(nc) as tc, ExitStack() as ctx:
        wpool = ctx.enter_context(tc.tile_pool(name="weights", bufs=1))
        cpool = ctx.enter_context(tc.tile_pool(name="consts", bufs=1))

        # ---- persistent weights ----
        wq_sb = wpool.tile([128, KD, D], BF16)
        nc.scalar.dma_start(wq_sb[:], d_wq[:].rearrange("(i p) m -> p i m", p=128))
        wk_sb = wpool.tile([128, KTD, D], BF16)
        nc.scalar.dma_start(wk_sb[:], d_wk[:].rearrange("(i p) m -> p i m", p=128))
        wv_sb = wpool.tile([128, KTD, D], BF16)
        nc.scalar.dma_start(wv_sb[:], d_wv[:].rearrange("(i p) m -> p i m", p=128))
        wo_sb = wpool.tile([128, KD, D], BF16)
        nc.scalar.dma_start(wo_sb[:], d_wo[:].rearrange("(i p) m -> p i m", p=128))

        # ---- constants ----
        bqc = cpool.tile([128, KD], F32)
        nc.scalar.dma_start(bqc[:], d_bqc[:])
        sel16 = cpool.tile([128, KD, H], BF16)
        nc.scalar.dma_start(sel16[:], d_sel16[:])
        pick = cpool.tile([H, KD, 128], BF16)
        nc.scalar.dma_start(pick[:], d_pick[:])
        bke_r = cpool.tile([1, D], BF16)
        nc.scalar.dma_start(bke_r[:], d_bke[None, :])
        bve_r = cpool.tile([1, D], BF16)
        nc.scalar.dma_start(bve_r[:], d_bve[None, :])
        ones_r = cpool.tile([1, 128], BF16)
        nc.vector.memset(ones_r[:], 1.0)
        ones_c = cpool.tile([128, 1], BF16)
        nc.vector.memset(ones_c[:], 1.0)
        eps_c = cpool.tile([128, 1], F32)
        nc.vector.memset(eps_c[:], EPS)
        ident = cpool.tile([128, 128], BF16)
        make_identity(nc, ident[:])
        identf = cpool.tile([4, 4], F32)
        make_identity(nc, identf[:])
        identf128 = cpool.tile([128, 128], F32)
        make_identity(nc, identf128[:])

        a_col = cpool.tile([128, KD, BPC], F32)   # FiLM A columns
        b_col = cpool.tile([128, KD, BPC], F32)   # FiLM B columns

        # ---- emb / FiLM phase (all BPC batches at once) ----
        with tc.tile_pool(name="wemb", bufs=1) as wep, \
             tc.tile_pool(name="etmp", bufs=1) as ep, \
             tc.tile_pool(name="pse", bufs=2, space=bass.MemorySpace.PSUM) as pse:
            wemb_sb = wep.tile([128, KTE, D2], BF16)
            embb_r = ep.tile([1, D2], BF16)
            nc.scalar.dma_start(embb_r[:], d_embb[None, :])
            fg_c = ep.tile([128, KD], F32)
            nc.scalar.dma_start(fg_c[:], d_fgc[:])
            fb_c = ep.tile([128, KD], F32)
            nc.scalar.dma_start(fb_c[:], d_fbc[:])
            nc.scalar.dma_start(wemb_sb[:],
                              d_wemb[:].rearrange("(i p) m -> p i m", p=128))
            emb_sb = ep.tile([BPC, TE], F32)
            nc.scalar.dma_start(emb_sb[:], d_emb[:])
            semb = ep.tile([BPC, TE], BF16)
            nc.scalar.activation(semb[:], emb_sb[:], AF.Silu)
            embT = ep.tile([128, KTE, BPC], BF16)
            for c in range(KTE):
                pst = pse.tile([128, BPC], BF16, tag="pst")
                nc.tensor.transpose(pst[:], semb[:, c * 128:(c + 1) * 128],
                                    ident[0:BPC, 0:BPC])
                nc.vector.tensor_copy(embT[:, c, :], pst[:])
            e_sb = ep.tile([BPC, D2], F32)
            for chn in range(D2 // 512):
                pe = pse.tile([BPC, 512], F32, tag="pe")
                for kt in range(KTE):
                    nc.tensor.matmul(pe[:], embT[:, kt, :],
                                     wemb_sb[:, kt, chn * 512:(chn + 1) * 512],
                                     start=(kt == 0), stop=False)
                nc.tensor.matmul(pe[:], ones_r[0:1, 0:BPC],
                                 embb_r[0:1, chn * 512:(chn + 1) * 512],
                                 start=False, stop=True)
                nc.vector.tensor_copy(e_sb[:, chn * 512:(chn + 1) * 512], pe[:])
            # transpose scale/shift to columns, build A/B FiLM columns
            for j in range(KD):
                js = slice(j * 128, (j + 1) * 128)
                pts = pse.tile([128, BPC], F32, tag="pts")
                nc.tensor.transpose(pts[:], e_sb[0:BPC, js], identf[:])
                sT = ep.tile([128, BPC], F32, tag="sT")
                nc.vector.tensor_copy(sT[:], pts[:])
                pth = pse.tile([128, BPC], F32, tag="pts")
                nc.tensor.transpose(pth[:], e_sb[0:BPC, D + j * 128:D + (j + 1) * 128],
                                    identf[:])
                hT = ep.tile([128, BPC], F32, tag="hT")
                nc.vector.tensor_copy(hT[:], pth[:])
                nc.vector.tensor_scalar(a_col[:, j, :], sT[:], 1.0,
                                        fg_c[:, j:j + 1], ALU.add, ALU.mult)
                tmb = ep.tile([128, BPC], F32, tag="tmb")
                nc.vector.tensor_scalar(tmb[:], sT[:], 1.0,
                                        fb_c[:, j:j + 1], ALU.add, ALU.mult)
                nc.vector.tensor_add(b_col[:, j, :], tmb[:], hT[:])

        # ---- batch-phase pools ----
        xtp = ctx.enter_context(tc.tile_pool(name="xt", bufs=1))
        xntp = ctx.enter_context(tc.tile_pool(name="xnt", bufs=1))
        htp = ctx.enter_context(tc.tile_pool(name="ht", bufs=1))
        sqp = ctx.enter_context(tc.tile_pool(name="sq", bufs=2))
        rowp = ctx.enter_context(tc.tile_pool(name="rows", bufs=1))
        bcp = ctx.enter_context(tc.tile_pool(name="bc", bufs=1))
        bcyp = ctx.enter_context(tc.tile_pool(name="bcy", bufs=2))
        xfp = ctx.enter_context(tc.tile_pool(name="xf", bufs=1))
        kvp = ctx.enter_context(tc.tile_pool(name="kv", bufs=1))
        qp = ctx.enter_context(tc.tile_pool(name="q", bufs=1))
        ytp = ctx.enter_context(tc.tile_pool(name="yt", bufs=1))
        rbcp = ctx.enter_context(tc.tile_pool(name="rbc", bufs=2))
        tmpp = ctx.enter_context(tc.tile_pool(name="tmp", bufs=2))
        resp = ctx.enter_context(tc.tile_pool(name="res", bufs=2))
        outp = ctx.enter_context(tc.tile_pool(name="o", bufs=2))
        psq = ctx.enter_context(tc.tile_pool(name="psq", bufs=2, space=bass.MemorySpace.PSUM))
        psmid = ctx.enter_context(tc.tile_pool(name="psmid", bufs=2, space=bass.MemorySpace.PSUM))
        psst = ctx.enter_context(tc.tile_pool(name="psst", bufs=1, space=bass.MemorySpace.PSUM))
        psa = ctx.enter_context(tc.tile_pool(name="psa", bufs=1, space=bass.MemorySpace.PSUM))
        pso = ctx.enter_context(tc.tile_pool(name="pso", bufs=1, space=bass.MemorySpace.PSUM))

        inv_d = 1.0 / D
        inv_td = 1.0 / TD

        for b in range(BPC):
            # ========== x path: transpose, stats, normalize ==========
            xT = xtp.tile([128, KD, T], BF16, tag="xT")
            for j in range(KD):
                nc.sync.dma_start_transpose(xT[:, j, :],
                                            d_xbf[b, :, j * 128:(j + 1) * 128])
            rstd_xb = rowp.tile([1, T], BF16, tag="rstd_xb")
            nmr_xb = rowp.tile([1, T], BF16, tag="nmr_xb")
            xnT = xntp.tile([128, KD, T], BF16, tag="xnT")
            for chn in range(NCH):
                cs = slice(chn * 512, (chn + 1) * 512)
                s1x = psst.tile([1, 512], F32, tag="s1")
                s2x = psst.tile([1, 512], F32, tag="s2")
                for j in range(KD):
                    sq = sqp.tile([128, 512], BF16, tag="sqx")
                    nc.vector.tensor_mul(sq[:], xT[:, j, cs], xT[:, j, cs])
                    nc.tensor.matmul(s1x[:], ones_c[:], xT[:, j, cs],
                                     start=(j == 0), stop=(j == KD - 1))
                    nc.tensor.matmul(s2x[:], ones_c[:], sq[:],
                                     start=(j == 0), stop=(j == KD - 1))
                mu_x = rowp.tile([1, 512], F32, tag=f"r1c{chn}")
                nc.vector.tensor_scalar_mul(mu_x[:], s1x[:], inv_d)
                msq_x = rowp.tile([1, 512], F32, tag="r2")
                nc.vector.tensor_mul(msq_x[:], mu_x[:], mu_x[:])
                u_x = rowp.tile([1, 512], F32, tag=f"r3c{chn}")
                nc.vector.scalar_tensor_tensor(u_x[:], msq_x[:], -float(D), s2x[:],
                                               op0=ALU.mult, op1=ALU.add)
                nc.scalar.activation(rstd_xb[0:1, cs], u_x[:], AF.Abs_reciprocal_sqrt,
                                     bias=eps_c[0:1, :], scale=inv_d)
                nc.vector.scalar_tensor_tensor(nmr_xb[0:1, cs], mu_x[:], -1.0,
                                               rstd_xb[0:1, cs],
                                               op0=ALU.mult, op1=ALU.mult)
                rstd_bc = bcp.tile([128, 512], BF16, tag="rstd_bc")
                nc.gpsimd.partition_broadcast(rstd_bc[:], rstd_xb[0:1, cs],
                                              channels=128)
                nmr_bc = bcp.tile([128, 512], BF16, tag="nmr_bc")
                nc.gpsimd.partition_broadcast(nmr_bc[:], nmr_xb[0:1, cs],
                                              channels=128)
                for j in range(KD):
                    t1 = tmpp.tile([128, 512], BF16, tag="t1y")
                    nc.vector.tensor_mul(t1[:], xT[:, j, cs], rstd_bc[:])
                    nc.vector.tensor_add(xnT[:, j, cs], t1[:], nmr_bc[:])

            # ========== xf path ==========
            xfT = xfp.tile([128, KTD, N], BF16, tag="xfT")
            for kt in range(KTD):
                nc.sync.dma_start_transpose(xfT[:, kt, :],
                                            d_xfbf[b, :, kt * 128:(kt + 1) * 128])
            s1f = psst.tile([1, N], F32, tag="s1")
            s2f = psst.tile([1, N], F32, tag="s2")
            for kt in range(KTD):
                sqf = sqp.tile([128, N], BF16, tag="sqf")
                nc.vector.tensor_mul(sqf[:], xfT[:, kt, :], xfT[:, kt, :])
                nc.tensor.matmul(s1f[:], ones_c[:], xfT[:, kt, :],
                                 start=(kt == 0), stop=(kt == KTD - 1))
                nc.tensor.matmul(s2f[:], ones_c[:], sqf[:],
                                 start=(kt == 0), stop=(kt == KTD - 1))
            mu_f = rowp.tile([1, N], F32, tag="r1c0")
            nc.vector.tensor_scalar_mul(mu_f[:], s1f[:], inv_td)
            msq_f = rowp.tile([1, N], F32, tag="r2")
            nc.vector.tensor_mul(msq_f[:], mu_f[:], mu_f[:])
            u_f = rowp.tile([1, N], F32, tag="r3c0")
            nc.vector.scalar_tensor_tensor(u_f[:], msq_f[:], -float(TD), s2f[:],
                                           op0=ALU.mult, op1=ALU.add)
            rstd_fb = rowp.tile([1, N], BF16, tag="rstd_fb")
            nc.scalar.activation(rstd_fb[:], u_f[:], AF.Abs_reciprocal_sqrt,
                                 bias=eps_c[0:1, :], scale=inv_td)
            nmr_fb = rowp.tile([1, N], BF16, tag="nmr_fb")
            nc.vector.scalar_tensor_tensor(nmr_fb[:], mu_f[:], -1.0, rstd_fb[:],
                                           op0=ALU.mult, op1=ALU.mult)
            rstdf_bc = bcp.tile([128, N], BF16, tag="rstdf_bc")
            nc.gpsimd.partition_broadcast(rstdf_bc[:], rstd_fb[:], channels=128)
            nmrf_bc = bcp.tile([128, N], BF16, tag="nmrf_bc")
            nc.gpsimd.partition_broadcast(nmrf_bc[:], nmr_fb[:], channels=128)
            xfnT = xfp.tile([128, KTD, N], BF16, tag="xfnT")
            for kt in range(KTD):
                t1f = tmpp.tile([128, N], BF16, tag="t1y")
                nc.vector.tensor_mul(t1f[:], xfT[:, kt, :], rstdf_bc[:])
                nc.vector.tensor_add(xfnT[:, kt, :], t1f[:], nmrf_bc[:])

            # ---- K and V ----
            exp_k = kvp.tile([128, NT, D], BF16, tag="expk")
            v_sb = kvp.tile([128, NT, D], BF16, tag="vsb")
            for nt in range(NT):
                ns = slice(nt * 128, (nt + 1) * 128)
                for chn in range(NCH):
                    cs = slice(chn * 512, (chn + 1) * 512)
                    pk = psq.tile([128, 512], F32, tag="ps")
                    for kt in range(KTD):
                        nc.tensor.matmul(pk[:], xfnT[:, kt, ns], wk_sb[:, kt, cs],
                                         start=(kt == 0), stop=False)
                    nc.tensor.matmul(pk[:], ones_r[0:1, 0:128], bke_r[0:1, cs],
                                     start=False, stop=True)
                    nc.scalar.activation(exp_k[:, nt, cs], pk[:], AF.Exp)
                    pv = psq.tile([128, 512], F32, tag="ps")
                    for kt in range(KTD):
                        nc.tensor.matmul(pv[:], xfnT[:, kt, ns], wv_sb[:, kt, cs],
                                         start=(kt == 0), stop=False)
                    nc.tensor.matmul(pv[:], ones_r[0:1, 0:128], bve_r[0:1, cs],
                                     start=False, stop=True)
                    nc.vector.tensor_copy(v_sb[:, nt, cs], pv[:])

            # ---- S_k and attn ----
            pks = psa.tile([128, KD], F32, tag="skattn")
            for j in range(KD):
                for nt in range(NT):
                    nc.tensor.matmul(pks[:, j:j + 1],
                                     exp_k[:, nt, j * 128:(j + 1) * 128],
                                     ones_c[:], start=(nt == 0), stop=(nt == 1))
            r_k = rowp.tile([128, KD], F32, tag="rk")
            nc.vector.reciprocal(r_k[:], pks[:])

            patt = psa.tile([128, 512], F32, tag="skattn")
            for h in range(H):
                rp = slice((h % 2) * 64, (h % 2) * 64 + 64)
                cp = slice((h // 2) * 64, (h // 2) * 64 + 64)
                hs = slice(h * 64, (h + 1) * 64)
                for nt in range(NT):
                    nc.tensor.matmul(patt[rp, cp], exp_k[:, nt, hs],
                                     v_sb[:, nt, hs],
                                     start=(nt == 0), stop=(nt == 1))
            # block-diagonal per head pair: [0:64,0:64]=head 2j, [64:,64:]=head 2j+1
            attn_s = kvp.tile([128, KD, 128], BF16, tag="attns")
            nc.vector.memset(attn_s[:], 0.0)
            for j in range(KD):
                nc.vector.tensor_scalar_mul(attn_s[0:64, j, 0:64],
                                            patt[0:64, j * 64:(j + 1) * 64],
                                            r_k[0:64, j:j + 1])
                nc.vector.tensor_scalar_mul(attn_s[64:128, j, 64:128],
                                            patt[64:128, j * 64:(j + 1) * 64],
                                            r_k[64:128, j:j + 1])

            # ========== middle section, phase-grouped across t-chunks ==========
            exp_qT = qp.tile([128, KD, T], BF16, tag="expq")
            yT = ytp.tile([128, KD, T], BF16, tag="yT")
            hT = htp.tile([128, KD, T], BF16, tag="hT")
            # Q projection -> exp, transposed (both chunks; Exp ops adjacent)
            for ch2 in range(NCH):
                ts_ = slice(ch2 * 512, (ch2 + 1) * 512)
                for j in range(KD):
                    js = slice(j * 128, (j + 1) * 128)
                    pq = psq.tile([128, 512], F32, tag="ps")
                    for kt in range(KD):
                        nc.tensor.matmul(pq[:], wq_sb[:, kt, js],
                                         xnT[:, kt, ts_],
                                         start=(kt == 0), stop=(kt == KD - 1))
                    nc.scalar.activation(exp_qT[:, j, ts_], pq[:], AF.Exp,
                                         bias=bqc[:, j:j + 1])
            # softmax denominators + y + stats (both chunks)
            stat_ps = []
            for ch2 in range(NCH):
                ts_ = slice(ch2 * 512, (ch2 + 1) * 512)
                s_all = psq.tile([H, 512], F32, tag="ps")
                for j in range(KD):
                    nc.tensor.matmul(s_all[:], sel16[:, j, :], exp_qT[:, j, ts_],
                                     start=(j == 0), stop=(j == KD - 1))
                rs_all = rbcp.tile([H, 512], BF16, tag="rsall")
                with nc.allow_low_precision(reason="softmax recip in bf16 is fine"):
                    nc.vector.reciprocal(rs_all[:], s_all[:])
                s1y = psst.tile([1, 512], F32, tag="s1")
                s2y = psst.tile([1, 512], F32, tag="s2")
                for j in range(KD):
                    pbc = psmid.tile([128, 512], F32, tag="pm")
                    nc.tensor.matmul(pbc[:], pick[:, j, :], rs_all[:],
                                     start=True, stop=True)
                    bcs = rbcp.tile([128, 512], BF16, tag="bcs")
                    nc.vector.tensor_copy(bcs[:], pbc[:])
                    py = psmid.tile([128, 512], F32, tag="pm")
                    nc.tensor.matmul(py[:], attn_s[:, j, :], exp_qT[:, j, ts_],
                                     start=True, stop=True)
                    nc.vector.tensor_mul(yT[:, j, ts_], py[:], bcs[:])
                    sqy = sqp.tile([128, 512], BF16, tag="sqy")
                    nc.vector.tensor_mul(sqy[:], yT[:, j, ts_], yT[:, j, ts_])
                    nc.tensor.matmul(s1y[:], ones_c[:], yT[:, j, ts_],
                                     start=(j == 0), stop=(j == KD - 1))
                    nc.tensor.matmul(s2y[:], ones_c[:], sqy[:],
                                     start=(j == 0), stop=(j == KD - 1))
                # drain stat psums now (DVE only): mu and u = D*var
                mu_y = rowp.tile([1, 512], F32, tag=f"r1c{ch2}")
                nc.vector.tensor_scalar_mul(mu_y[:], s1y[:], inv_d)
                msq_y = rowp.tile([1, 512], F32, tag="r2")
                nc.vector.tensor_mul(msq_y[:], mu_y[:], mu_y[:])
                u_y = rowp.tile([1, 512], F32, tag=f"r3c{ch2}")
                nc.vector.scalar_tensor_tensor(u_y[:], msq_y[:], -float(D), s2y[:],
                                               op0=ALU.mult, op1=ALU.add)
                stat_ps.append((mu_y, u_y))
            # y LN rows + broadcasts (both chunks; arsqrt ops adjacent)
            ybcs = []
            for ch2 in range(NCH):
                mu_y, u_y = stat_ps[ch2]
                rstd_yb = rowp.tile([1, 512], BF16, tag="rstd_yb")
                nc.scalar.activation(rstd_yb[:], u_y[:], AF.Abs_reciprocal_sqrt,
                                     bias=eps_c[0:1, :], scale=inv_d)
                nmr_yb = rowp.tile([1, 512], BF16, tag="nmr_yb")
                nc.vector.scalar_tensor_tensor(nmr_yb[:], mu_y[:], -1.0, rstd_yb[:],
                                               op0=ALU.mult, op1=ALU.mult)
                rstdy_bc = bcyp.tile([128, 512], BF16, tag="rstdy_bc")
                nc.gpsimd.partition_broadcast(rstdy_bc[:], rstd_yb[:], channels=128)
                nmry_bc = bcyp.tile([128, 512], BF16, tag="nmry_bc")
                nc.gpsimd.partition_broadcast(nmry_bc[:], nmr_yb[:], channels=128)
                ybcs.append((rstdy_bc, nmry_bc))
            # apply LN + FiLM + silu (both chunks; Silu ops adjacent)
            for ch2 in range(NCH):
                ts_ = slice(ch2 * 512, (ch2 + 1) * 512)
                rstdy_bc, nmry_bc = ybcs[ch2]
                for j in range(KD):
                    t1 = tmpp.tile([128, 512], BF16, tag="t1y")
                    nc.vector.tensor_mul(t1[:], yT[:, j, ts_], rstdy_bc[:])
                    t2 = tmpp.tile([128, 512], BF16, tag="t2y")
                    nc.vector.tensor_add(t2[:], t1[:], nmry_bc[:])
                    nc.scalar.activation(hT[:, j, ts_], t2[:], AF.Silu,
                                         bias=b_col[:, j, b:b + 1],
                                         scale=a_col[:, j, b:b + 1])
            for ch2 in range(NCH):
                # out projection + residual for the 4 t-tiles of this chunk
                for tti in range(4):
                    ti = ch2 * 4 + tti
                    trs = slice(ti * 128, (ti + 1) * 128)
                    for chn in range(NCH):
                        cs = slice(chn * 512, (chn + 1) * 512)
                        xr = resp.tile([128, 512], F32, tag="xr")
                        nc.scalar.dma_start(xr[:], d_xf32[b, trs, cs])
                        po = pso.tile([128, 512], F32, tag="po")
                        for j in range(KD):
                            nc.tensor.matmul(po[:], hT[:, j, trs], wo_sb[:, j, cs],
                                             start=(j == 0), stop=(j == KD - 1))
                        o_sb = outp.tile([128, 512], F32, tag="osb")
                        nc.vector.tensor_add(o_sb[:], po[:], xr[:])
                        nc.gpsimd.dma_start(d_out[b, trs, cs], o_sb[:])

    nc.compile()
    return nc


def _get_program():
    global _PROGRAM
    if _PROGRAM is None:
        _PROGRAM = _build_program()
    return _PROGRAM


def _prep_inputs(inputs):
    f = lambda k: np.asarray(inputs[k], np.float32)
    x, xf, emb = f("x"), f("xf"), f("emb")
    norm_g, norm_b = f("norm_g"), f("norm_b")
    tnorm_g, tnorm_b = f("tnorm_g"), f("tnorm_b")
    Wq, bq, Wk, bk, Wv, bv = f("Wq"), f("bq"), f("Wk"), f("bk"), f("Wv"), f("bv")
    emb_W, emb_b = f("emb_W"), f("emb_b")
    fg, fb = f("fnorm_g"), f("fnorm_b")
    out_W, out_b = f("out_W"), f("out_b")

    wq_e = norm_g[:, None] * Wq
    wk_e = tnorm_g[:, None] * Wk
    wv_e = tnorm_g[:, None] * Wv
    bq_eff = bq + norm_b @ Wq          # [D]
    sel16 = np.zeros((128, KD, H), np.float32)
    pick = np.zeros((H, KD, 128), np.float32)
    for j in range(KD):
        sel16[0:64, j, 2 * j] = 1.0
        sel16[64:128, j, 2 * j + 1] = 1.0
        pick[2 * j, j, 0:64] = 1.0
        pick[2 * j + 1, j, 64:128] = 1.0
    shared = {
        "wq": wq_e.astype(NBF), "wk": wk_e.astype(NBF), "wv": wv_e.astype(NBF),
        "wo": out_W.astype(NBF), "wemb": emb_W.astype(NBF),
        "bqc": np.ascontiguousarray(bq_eff.reshape(KD, 128).T),
        "bke": (bk + tnorm_b @ Wk).astype(NBF),
        "bve": (bv + tnorm_b @ Wv).astype(NBF),
        "embb": emb_b.astype(NBF),
        "fgc": np.ascontiguousarray(fg.reshape(KD, 128).T),
        "fbc": np.ascontiguousarray(fb.reshape(KD, 128).T),
        "sel16": sel16.astype(NBF), "pick": pick.astype(NBF),
    }
    xbf = x.astype(NBF)
    xfbf = xf.astype(NBF)
    xres = x + out_b[None, None, :]
    in_maps = []
    for i in range(NCORES):
        s = slice(i * BPC, (i + 1) * BPC)
        m = dict(shared)
        m["xbf"] = xbf[s]
        m["xf32"] = xres[s]
        m["xfbf"] = xfbf[s]
        m["emb"] = emb[s]
        in_maps.append(m)
    return in_maps


def run(inputs, trace=False):
    nc = _get_program()
    in_maps = _prep_inputs(inputs)
    res = run_bass_kernel_spmd(nc, in_maps, core_ids=list(range(NCORES)),
                               trace=trace)
    out = np.concatenate([res.results[i]["out"] for i in range(NCORES)], axis=0)
    return out, res


def kernel(**inputs):
    out, _ = run(inputs, trace=False)
    return out
